# revision 1
# baseline (speedup 1.0000x reference)
"""GAU (Gated Attention Unit) layer kernel for Trainium2, 8 NeuronCores.

Sharding: query-sequence-parallel within batch. 4 batches x 2 query slabs
of 2048 -> 8 cores. Each core receives the full 4096-token sequence of its
batch (rows reordered so its own query slab comes first), computes the
full-sequence K/V projection, and attention + output projection for its
own 2048 queries.

Per-core dataflow (all matmuls in bf16, fp32 PSUM accumulation):
  1a. transpose h [tok,feat] -> hT [feat,tok] via PE transposes (bf16 out)
  1b. v = silu(h@Wi_v + bi_v)  token-major [tok, 1536], resident in SBUF
  1c. qk = silu(h@Wi_qk + bi_qk) feature-major; gamma/beta + RoPE -> qT,kT
      (qk columns of Wi are host-permuted evens-first so RoPE halves are
      contiguous partition ranges; the 1/sqrt(d*n) score scale is folded
      into q_gamma/q_beta host-side)
  1d. u = silu(h@Wi_u + bi_u) feature-major, spilled to DRAM scratch
  2.  per 512-query chunk: scores^T = kT.T@qT -> A = relu(s)^2 (bf16),
      Av^T = v.T@A accumulated over 32 key tiles, g = u * Av^T,
      out = g.T@Wo + bo + h residual, RMS-normalize, DMA out.

silu is emitted as x*sigmoid(x) (Silu has no ACT table in this stack).
"""

import os

import ml_dtypes
import numpy as np

import concourse.bass as bass
import concourse.mybir as mybir
import concourse.tile as tile
from concourse import bacc, bass_utils

P = 128
SEQ = 4096
DIM = 768
NCOL = 3200
UV = 1536
KEY = 128
HALF = 64
SLAB = 2048
KD = DIM // P        # 6 feature k-tiles
KT = SEQ // P        # 32 key-token tiles
CH = 512
NCH = SEQ // CH      # 8 token chunks
OWN_CH = SLAB // CH  # 4 own (query) chunks
VT = UV // CH        # 3 v-column chunks
UT = UV // P         # 12 u/v feature tiles
NB = 4
NCORES = 8
EPS = 1e-12

F32 = mybir.dt.float32
BF16 = mybir.dt.bfloat16
F8 = mybir.dt.float8e4
OP = mybir.AluOpType
AF = mybir.ActivationFunctionType

_cache = {}
LAST_RESULT = None

CFG = {
    "ps_t": 4, "ps_p": 3,
    "ps_s": 2, "ps_av": 2, "ps_o": 2,
    "p2g": 1, "p2at": 1,
}


def _build(upto=7, cfg=None):
    cfg = {**CFG, **(cfg or {})}
    nc = bacc.Bacc(
        "TRN2", target_bir_lowering=False, debug=False, num_devices=NCORES
    )

    def din(name, shape, dt):
        return nc.dram_tensor(name, list(shape), dt, kind="ExternalInput").ap()

    h_d = din("h", [SEQ, DIM], F32)
    wi_d = din("wi", [DIM, NCOL], F32)
    wo_d = din("wo", [UV, DIM], F32)
    bi_u_d = din("bi_u8", [1, UV], F8)
    bi_v_d = din("bi_v8", [1, UV], F8)
    bi_qk_d = din("bi_qk8", [1, P], F8)
    bo_d = din("bo", [1, DIM], BF16)
    gb_d = din("gb", [P, 4], F32)
    cc_d = din("cc", [P, SEQ], BF16)
    ss_d = din("ss", [P, SEQ], BF16)
    id_d = din("ident", [P, P], F32)
    out_d = nc.dram_tensor("out", [SLAB, DIM], F32, kind="ExternalOutput").ap()
    dbg_d = None
    if upto < 7:
        dbg_d = nc.dram_tensor("dbg", [P, SEQ], BF16, kind="ExternalOutput").ap()

    with tile.TileContext(nc) as tc:
        with (
            tc.tile_pool(name="consts", bufs=1) as consts,
            tc.tile_pool(name="persist", bufs=1) as persist,
            tc.tile_pool(name="dram", bufs=1, space="DRAM") as dram,
        ):
            gb_sb = consts.tile([P, 4], F32, tag="gb", name="gb_sb")
            bi_u_sb = consts.tile([1, UV], F8, tag="biu", name="bi_u_sb")
            bi_qk_sb = consts.tile([1, P], F8, tag="biqk", name="bi_qk_sb")
            bi_v_sb = consts.tile([1, UV], F8, tag="biv", name="bi_v_sb")
            bo_sb = consts.tile([1, DIM], BF16, tag="bo", name="bo_sb")
            ones_sb = consts.tile([1, P], BF16, tag="ones", name="ones_sb")
            ones8_sb = consts.tile([1, CH], F8, tag="ones8", name="ones8_sb")
            id_sb = consts.tile([P, P], F32, tag="id", name="id_sb")
            eps_sb = consts.tile([P, 1], F32, tag="eps", name="eps_sb")

            nc.sync.dma_start(out=gb_sb, in_=gb_d)
            nc.sync.dma_start(out=bi_u_sb, in_=bi_u_d)
            nc.sync.dma_start(out=bi_qk_sb, in_=bi_qk_d)
            nc.sync.dma_start(out=bi_v_sb, in_=bi_v_d)
            nc.sync.dma_start(out=bo_sb, in_=bo_d)
            nc.sync.dma_start(out=id_sb, in_=id_d)
            nc.vector.memset(ones_sb, 1.0)
            nc.vector.memset(ones8_sb, 1.0)
            nc.vector.memset(eps_sb, EPS)

            v_sb = persist.tile([P, KT, UV], F8, tag="v", name="v_sb")
            kT_sb = persist.tile([P, SEQ], BF16, tag="kT", name="kT_sb")
            qT_sb = persist.tile([P, SLAB], BF16, tag="qT", name="qT_sb")
            u_sb = persist.tile([P, UT, SLAB], BF16, tag="u", name="u_sb")

            # ---------------- Phase 1: projections ----------------
            with (
                tc.tile_pool(name="p1ht", bufs=1) as p1ht,
                tc.tile_pool(name="p1sb", bufs=3) as p1sb,
                tc.tile_pool(name="p1w", bufs=2) as p1w,
                tc.tile_pool(name="p1cs", bufs=1) as p1cs,
                tc.tile_pool(name="ps_t", bufs=cfg["ps_t"], space="PSUM") as ps_t,
                tc.tile_pool(name="ps_p", bufs=cfg["ps_p"], space="PSUM") as ps_p,
            ):
                # 1a: full hT, feature-major bf16 [P, KD, SEQ]
                hT = p1ht.tile([P, KD, SEQ], F8, tag="hT", name="hT")
                for tt in range(KT):
                    hst = p1sb.tile([P, DIM], F32, tag="hst", name="hst", bufs=2)
                    nc.sync.dma_start(out=hst, in_=h_d[tt * P:(tt + 1) * P, :])
                    for kd in range(KD):
                        tp = ps_t.tile([P, P], F32, tag="tp", name="tp")
                        nc.tensor.transpose(tp, hst[:, kd * P:(kd + 1) * P], id_sb)
                        nc.vector.tensor_copy(
                            out=hT[:, kd, tt * P:(tt + 1) * P], in_=tp
                        )
                if upto == 1:
                    nc.sync.dma_start(out=dbg_d, in_=hT[:, 0, :])

                # 1b: v token-major, full sequence
                if upto >= 2:
                    for vc in range(VT):
                        wvc = p1w.tile(
                            [P, KD, CH], F8, tag="wvc", name="wvc", bufs=1
                        )
                        for kd in range(KD):
                            wst = p1sb.tile(
                                [P, CH], F32, tag="wst", name="wst", bufs=2
                            )
                            nc.sync.dma_start(
                                out=wst,
                                in_=wi_d[kd * P:(kd + 1) * P,
                                         UV + vc * CH:UV + (vc + 1) * CH],
                            )
                            nc.vector.tensor_copy(out=wvc[:, kd, :], in_=wst)
                        for tt in range(KT):
                            pv = ps_p.tile([P, CH], F32, tag="pp", name="pv")
                            nc.tensor.matmul(
                                pv, ones8_sb[:, 0:P],
                                bi_v_sb[:, vc * CH:(vc + 1) * CH],
                                start=True, stop=False,
                            )
                            for kd2 in range(KD // 2):
                                nc.tensor.matmul(
                                    pv,
                                    hT[:, 2 * kd2:2 * kd2 + 2,
                                       tt * P:(tt + 1) * P],
                                    wvc[:, 2 * kd2:2 * kd2 + 2, :],
                                    start=False, stop=(kd2 == KD // 2 - 1),
                                    perf_mode=mybir.MatmulPerfMode.DoubleRow,
                                )
                            sg = p1sb.tile(
                                [P, CH], BF16, tag="sg", name="sg", bufs=2
                            )
                            nc.scalar.activation(
                                out=sg, in_=pv, func=AF.Sigmoid, scale=1.0 / 16
                            )
                            nc.vector.scalar_tensor_tensor(
                                out=v_sb[:, tt, vc * CH:(vc + 1) * CH], in0=pv,
                                scalar=1.0 / 16, in1=sg,
                                op0=OP.mult, op1=OP.mult,
                            )
                    if upto == 2:
                        nc.sync.dma_start(
                            out=dbg_d[:, 0:UV], in_=v_sb[:, 0, :]
                        )

                # 1c: qk feature-major + gamma/beta + rope -> kT (all), qT (own)
                if upto >= 3:
                    wqk = p1w.tile(
                        [P, KD, KEY], F8, tag="wqk", name="wqk", bufs=1
                    )
                    for kd in range(KD):
                        wst = p1sb.tile(
                            [P, KEY], F32, tag="wstq", name="wstq", bufs=2
                        )
                        nc.sync.dma_start(
                            out=wst, in_=wi_d[kd * P:(kd + 1) * P, 2 * UV:NCOL]
                        )
                        nc.vector.tensor_copy(out=wqk[:, kd, :], in_=wst)
                    for ch in range(NCH):
                        t0 = ch * CH
                        pq = ps_p.tile([P, CH], F32, tag="pp", name="pq")
                        nc.tensor.matmul(
                            pq, bi_qk_sb, ones8_sb, start=True, stop=False
                        )
                        for kd2 in range(KD // 2):
                            nc.tensor.matmul(
                                pq, wqk[:, 2 * kd2:2 * kd2 + 2, :],
                                hT[:, 2 * kd2:2 * kd2 + 2, t0:t0 + CH],
                                start=False, stop=(kd2 == KD // 2 - 1),
                                perf_mode=mybir.MatmulPerfMode.DoubleRow,
                            )
                        qk_f = p1sb.tile(
                            [P, CH], BF16, tag="qkf", name="qk_f", bufs=2
                        )
                        sgq = p1sb.tile([P, CH], BF16, tag="sg", name="sgq",
                                        bufs=2)
                        nc.scalar.activation(
                            out=sgq, in_=pq, func=AF.Sigmoid, scale=1.0 / 16
                        )
                        nc.vector.scalar_tensor_tensor(
                            out=qk_f, in0=pq, scalar=1.0 / 16, in1=sgq,
                            op0=OP.mult, op1=OP.mult,
                        )
                        cct = p1cs.tile([P, CH], BF16, tag="cct", name="cct",
                                        bufs=2)
                        sst = p1cs.tile([P, CH], BF16, tag="sst", name="sst",
                                        bufs=2)
                        nc.sync.dma_start(out=cct, in_=cc_d[:, t0:t0 + CH])
                        nc.sync.dma_start(out=sst, in_=ss_d[:, t0:t0 + CH])
                        targets = [(kT_sb[:, t0:t0 + CH], 2)]
                        if ch < OWN_CH:
                            targets.append((qT_sb[:, t0:t0 + CH], 0))
                        for dst, gi in targets:
                            pre = p1sb.tile(
                                [P, CH], F32, tag="pre", name="pre", bufs=2
                            )
                            nc.vector.tensor_scalar(
                                out=pre, in0=qk_f,
                                scalar1=gb_sb[:, gi:gi + 1],
                                scalar2=gb_sb[:, gi + 1:gi + 2],
                                op0=OP.mult, op1=OP.add,
                            )
                            x1 = pre[0:HALF, :]
                            x2 = pre[HALF:P, :]
                            ta = p1sb.tile([HALF, CH], BF16, tag="ta",
                                           name="ta", bufs=2)
                            tb = p1sb.tile([HALF, CH], BF16, tag="tb",
                                           name="tb", bufs=2)
                            nc.vector.tensor_mul(
                                out=ta, in0=x1, in1=cct[0:HALF, :]
                            )
                            nc.vector.tensor_mul(
                                out=tb, in0=x2, in1=sst[HALF:P, :]
                            )
                            nc.vector.tensor_sub(
                                out=dst[0:HALF, :], in0=ta, in1=tb
                            )
                            tc_ = p1sb.tile([HALF, CH], BF16, tag="ta",
                                            name="tc_", bufs=2)
                            td = p1sb.tile([HALF, CH], BF16, tag="tb",
                                           name="td", bufs=2)
                            nc.vector.tensor_mul(
                                out=tc_, in0=x1, in1=sst[0:HALF, :]
                            )
                            nc.vector.tensor_mul(
                                out=td, in0=x2, in1=cct[HALF:P, :]
                            )
                            nc.vector.tensor_add(
                                out=dst[HALF:P, :], in0=tc_, in1=td
                            )
                    if upto == 3:
                        nc.sync.dma_start(out=dbg_d, in_=kT_sb)

                # 1d: u feature-major for own tokens, spill to DRAM
                if upto >= 4:
                    for ut in range(UT):
                        wuc = p1w.tile([P, KD, P], F8, tag="wuc", name="wuc")
                        for kd in range(KD):
                            wst = p1sb.tile(
                                [P, P], F32, tag="wstu", name="wstu", bufs=2
                            )
                            nc.sync.dma_start(
                                out=wst,
                                in_=wi_d[kd * P:(kd + 1) * P,
                                         ut * P:(ut + 1) * P],
                            )
                            nc.vector.tensor_copy(out=wuc[:, kd, :], in_=wst)
                        for qc in range(OWN_CH):
                            t0 = qc * CH
                            pu = ps_p.tile([P, CH], F32, tag="pp", name="pu")
                            nc.tensor.matmul(
                                pu, bi_u_sb[:, ut * P:(ut + 1) * P],
                                ones8_sb,
                                start=True, stop=False,
                            )
                            for kd2 in range(KD // 2):
                                nc.tensor.matmul(
                                    pu, wuc[:, 2 * kd2:2 * kd2 + 2, :],
                                    hT[:, 2 * kd2:2 * kd2 + 2, t0:t0 + CH],
                                    start=False, stop=(kd2 == KD // 2 - 1),
                                    perf_mode=mybir.MatmulPerfMode.DoubleRow,
                                )
                            sgu = p1sb.tile(
                                [P, CH], BF16, tag="sg", name="sgu", bufs=2
                            )
                            nc.scalar.activation(
                                out=sgu, in_=pu, func=AF.Sigmoid, scale=1.0 / 16
                            )
                            nc.vector.scalar_tensor_tensor(
                                out=u_sb[:, ut, t0:t0 + CH], in0=pu,
                                scalar=1.0 / 16,
                                in1=sgu, op0=OP.mult, op1=OP.mult,
                            )
                    if upto == 4:
                        nc.sync.dma_start(
                            out=dbg_d[:, 0:SLAB], in_=u_sb[:, 0, :]
                        )

            # ---------------- Phase 2: attention + output ----------------
            if upto >= 5:
                with (
                    tc.tile_pool(name="p2wo", bufs=1) as p2wo,
                    tc.tile_pool(name="p2at", bufs=cfg["p2at"]) as p2at,
                    tc.tile_pool(name="p2g", bufs=cfg["p2g"]) as p2g,
                    tc.tile_pool(name="p2sb", bufs=3) as p2sb,
                    tc.tile_pool(name="ps_s", bufs=cfg["ps_s"], space="PSUM") as ps_s,
                    tc.tile_pool(name="ps_av", bufs=cfg["ps_av"], space="PSUM") as ps_av,
                    tc.tile_pool(name="ps_o", bufs=cfg["ps_o"], space="PSUM") as ps_o,
                ):
                    wo_sb = p2wo.tile([P, UT, DIM], BF16, tag="wo", name="wo_sb")
                    for ut in range(UT):
                        wst = p2sb.tile(
                            [P, DIM], F32, tag="wsto", name="wsto", bufs=2
                        )
                        nc.sync.dma_start(
                            out=wst, in_=wo_d[ut * P:(ut + 1) * P, :]
                        )
                        nc.vector.tensor_copy(out=wo_sb[:, ut, :], in_=wst)

                    for qc in range(OWN_CH):
                        q0 = qc * CH
                        at = p2at.tile([P, KT, CH], F8, tag="at", name="at")
                        for kt in range(KT):
                            ps = ps_s.tile([P, CH], F32, tag="ps", name="ps")
                            nc.tensor.matmul(
                                ps, kT_sb[:, kt * P:(kt + 1) * P],
                                qT_sb[:, q0:q0 + CH], start=True, stop=True,
                            )
                            # A = relu(s)^2: ACT relu from PSUM, DVE square
                            rl = p2sb.tile(
                                [P, CH], BF16, tag="rl", name="rl", bufs=3
                            )
                            nc.scalar.activation(out=rl, in_=ps, func=AF.Relu)
                            nc.vector.tensor_mul(
                                out=at[:, kt, :], in0=rl, in1=rl
                            )
                        if upto == 5:
                            if qc == 0:
                                nc.sync.dma_start(
                                    out=dbg_d, in_=at[:, 0:NCH, :]
                                )
                            continue
                        g_sb = p2g.tile([P, UT, CH], BF16, tag="g", name="g_sb")
                        for ut in range(UT):
                            pav = ps_av.tile([P, CH], F32, tag="pav", name="pav")
                            for kt2 in range(KT // 2):
                                nc.tensor.matmul(
                                    pav,
                                    v_sb[:, 2 * kt2:2 * kt2 + 2,
                                         ut * P:(ut + 1) * P],
                                    at[:, 2 * kt2:2 * kt2 + 2, :],
                                    start=(kt2 == 0),
                                    stop=(kt2 == KT // 2 - 1),
                                    perf_mode=mybir.MatmulPerfMode.DoubleRow,
                                )
                            # scores carry only 1/sqrt(d); fold 1/SEQ here
                            nc.vector.scalar_tensor_tensor(
                                out=g_sb[:, ut, :], in0=pav,
                                scalar=1.0 / SEQ,
                                in1=u_sb[:, ut, q0:q0 + CH],
                                op0=OP.mult, op1=OP.mult,
                            )
                        if upto == 6:
                            if qc == 0:
                                nc.sync.dma_start(
                                    out=dbg_d, in_=g_sb[:, 0:NCH, :]
                                )
                            continue
                        for t in range(4):
                            tok0 = q0 + t * P
                            po_a = ps_o.tile([P, CH], F32, tag="poa",
                                             name="po_a")
                            po_b = ps_o.tile([P, DIM - CH], F32, tag="pob",
                                             name="po_b")
                            nc.tensor.matmul(
                                po_a, ones_sb, bo_sb[:, 0:CH],
                                start=True, stop=False,
                            )
                            nc.tensor.matmul(
                                po_b, ones_sb, bo_sb[:, CH:DIM],
                                start=True, stop=False,
                            )
                            for ut in range(UT):
                                g_t = g_sb[:, ut, t * P:(t + 1) * P]
                                nc.tensor.matmul(
                                    po_a, g_t, wo_sb[:, ut, 0:CH],
                                    start=False, stop=(ut == UT - 1),
                                )
                                nc.tensor.matmul(
                                    po_b, g_t, wo_sb[:, ut, CH:DIM],
                                    start=False, stop=(ut == UT - 1),
                                )
                            hres = p2sb.tile(
                                [P, DIM], F32, tag="hres", name="hres", bufs=2
                            )
                            nc.sync.dma_start(
                                out=hres, in_=h_d[tok0:tok0 + P, :]
                            )
                            o_sb = p2sb.tile(
                                [P, DIM], F32, tag="osb", name="o_sb", bufs=2
                            )
                            nc.vector.tensor_add(
                                out=o_sb[:, 0:CH], in0=po_a, in1=hres[:, 0:CH]
                            )
                            nc.vector.tensor_add(
                                out=o_sb[:, CH:DIM], in0=po_b,
                                in1=hres[:, CH:DIM],
                            )
                            ofin = p2sb.tile(
                                [P, DIM], F32, tag="ofin", name="ofin", bufs=2
                            )
                            # mean(o^2) via ACT Square + accum
                            # (tensor_tensor_reduce crashes the exec unit)
                            o2 = p2sb.tile(
                                [P, DIM], BF16, tag="o2", name="o2", bufs=1
                            )
                            ms = p2sb.tile([P, 1], F32, tag="ms", name="ms")
                            nc.scalar.activation(
                                out=o2, in_=o_sb, func=AF.Square,
                                accum_out=ms,
                            )
                            sd = p2sb.tile([P, 1], F32, tag="sd", name="sd")
                            nc.scalar.activation(
                                out=sd, in_=ms, func=AF.Sqrt,
                                bias=eps_sb[:, 0:1], scale=1.0 / DIM,
                            )
                            rinv = p2sb.tile([P, 1], F32, tag="rinv",
                                             name="rinv")
                            nc.vector.reciprocal(out=rinv, in_=sd)
                            nc.vector.tensor_scalar_mul(
                                ofin, o_sb, rinv[:, 0:1]
                            )
                            nc.sync.dma_start(
                                out=out_d[tok0:tok0 + P, :], in_=ofin
                            )
    nc.compile()
    return nc


def _get_nc(upto=7):
    key = ("nc", upto)
    if key not in _cache:
        _cache[key] = _build(upto)
    return _cache[key]


def _host_prep(hidden_states, Wi, bi, Wo, bo, q_gamma, q_beta, k_gamma, k_beta):
    h = np.ascontiguousarray(np.asarray(hidden_states, dtype=np.float32))
    Wi = np.asarray(Wi, dtype=np.float32)
    bi = np.asarray(bi, dtype=np.float32)
    Wo = np.ascontiguousarray(np.asarray(Wo, dtype=np.float32))
    bo = np.asarray(bo, dtype=np.float32)

    perm = np.concatenate([np.arange(0, KEY, 2), np.arange(1, KEY, 2)])
    Wi_p = Wi.copy()
    Wi_p[:, 2 * UV:] = Wi_p[:, 2 * UV:][:, perm]
    Wi_p *= 16.0  # scale into e4m3 normal range; 1/16 applied after psum

    c = float(KEY ** -0.5)
    gb = np.stack(
        [
            np.asarray(q_gamma, np.float32)[perm] * c,
            np.asarray(q_beta, np.float32)[perm] * c,
            np.asarray(k_gamma, np.float32)[perm],
            np.asarray(k_beta, np.float32)[perm],
        ],
        axis=1,
    ).astype(np.float32)

    bi_u = (16.0 * bi[:UV]).reshape(1, UV).astype(ml_dtypes.float8_e4m3)
    bi_v = (16.0 * bi[UV:2 * UV]).reshape(1, UV).astype(ml_dtypes.float8_e4m3)
    bi_qk = (16.0 * bi[2 * UV:][perm]).reshape(1, P).astype(
        ml_dtypes.float8_e4m3)

    omega = 1.0 / (10000.0 ** (np.arange(HALF, dtype=np.float32) / HALF))
    ang = np.arange(SEQ, dtype=np.float32)[:, None] * omega[None, :]
    cos_t = np.cos(ang).T
    sin_t = np.sin(ang).T
    cc_full = np.concatenate([cos_t, cos_t], axis=0).astype(ml_dtypes.bfloat16)
    ss_full = np.concatenate([sin_t, sin_t], axis=0).astype(ml_dtypes.bfloat16)

    shared = {
        "wi": Wi_p,
        "wo": Wo,
        "bi_u8": bi_u,
        "bi_v8": bi_v,
        "bi_qk8": bi_qk,
        "bo": bo.reshape(1, DIM).astype(ml_dtypes.bfloat16),
        "gb": gb,
        "ident": np.eye(P, dtype=np.float32),
    }
    in_maps = []
    for core in range(NCORES):
        b, s = divmod(core, 2)
        order = np.concatenate(
            [
                np.arange(s * SLAB, (s + 1) * SLAB),
                np.arange((1 - s) * SLAB, (2 - s) * SLAB),
            ]
        )
        m = dict(shared)
        m["h"] = np.ascontiguousarray(h[b][order])
        m["cc"] = np.ascontiguousarray(cc_full[:, order])
        m["ss"] = np.ascontiguousarray(ss_full[:, order])
        in_maps.append(m)
    return in_maps


def kernel(hidden_states, Wi, bi, Wo, bo, q_gamma, q_beta, k_gamma, k_beta):
    global LAST_RESULT
    nc = _get_nc()
    in_maps = _host_prep(
        hidden_states, Wi, bi, Wo, bo, q_gamma, q_beta, k_gamma, k_beta
    )
    res = bass_utils.run_bass_kernel_spmd(
        nc,
        in_maps,
        core_ids=list(range(NCORES)),
        trace=bool(int(os.environ.get("KTRACE", "0"))),
    )
    LAST_RESULT = res
    out = np.empty((NB, SEQ, DIM), dtype=np.float32)
    for core in range(NCORES):
        b, s = divmod(core, 2)
        out[b, s * SLAB:(s + 1) * SLAB] = res.results[core]["out"]
    return out



# revision 18
# speedup vs baseline: 1.4468x; 1.4468x over previous
"""GAU (Gated Attention Unit) layer kernel for Trainium2, 8 NeuronCores.

Sharding: query-sequence-parallel within batch. 4 batches x 2 query slabs
of 2048 -> 8 cores. Each core receives the full 4096-token sequence of its
batch (rows reordered so its own query slab comes first), computes the
full-sequence K/V projection, and attention + output projection for its
own 2048 queries.

v2: all heavy lifting pre-staged on host (h pre-transposed + cast fp8,
Wi/Wo pre-cast fp8 with x16 scale), silu on the ACT engine (the silu
table exists on TRN2 even though CoreSim lacks it), per-partition biases
via the ACT bias path, fp8 DoubleRow output projection, bo folded into
the residual h on host. Per-core dataflow (fp32 PSUM accumulation):
  1a. qk = silu(h@Wi_qk + b) feature-major; gamma/beta + RoPE -> qT,kT
      (qk columns host-permuted evens-first; 1/sqrt(d) folded into
      q_gamma/q_beta host-side)
  1b. v = silu(h@Wi_v + b) token-major [tok,1536] fp8 (bias via ones
      matmul into PSUM; silu in one ACT op per 128-token row)
  1c. u = silu(h@Wi_u + b) feature-major fp8, bias via ACT bias
  2.  two query-pair phases (qph x 1024 tokens): scores^T = kT.T@qT,
      at = relu(s)^2 (ACT relu + DVE square, fp8), Av^T accumulated
      over 32 key tiles fp8-DR, g = u * Av^T fp8, out = g@Wo fp8-DR,
      o = po/65536 + (h+bo), RMS-normalize, DMA out.
"""

import os

import ml_dtypes
import numpy as np

import concourse.bass as bass
import concourse.mybir as mybir
import concourse.tile as tile
from concourse import bacc, bass_utils

P = 128
SEQ = 4096
DIM = 768
NCOL = 3200
UV = 1536
KEY = 128
HALF = 64
SLAB = 2048
KD = DIM // P        # 6 feature k-tiles
KD2 = KD // 2        # 3 DoubleRow feature pairs
KT = SEQ // P        # 32 key-token tiles
KT2 = KT // 2        # 16 DoubleRow key pairs
CH = 512
NCH = SEQ // CH      # 8 token chunks
OWN_CH = SLAB // CH  # 4 own (query) chunks
VT = UV // CH        # 3 v-column chunks
UT = UV // P         # 12 u/v feature tiles
UT2 = UT // 2        # 6 DoubleRow u pairs
NB = 4
NCORES = 8
EPS = 1e-12
QPH = 2              # query-pair phases
QPW = SLAB // QPH    # 1024 tokens per phase
OSC = 1.0 / (16.0 * SEQ)  # output descale: wo x16, at carries xSEQ

F32 = mybir.dt.float32
BF16 = mybir.dt.bfloat16
F8 = mybir.dt.float8e4
OP = mybir.AluOpType
AF = mybir.ActivationFunctionType
DR = mybir.MatmulPerfMode.DoubleRow

_cache = {}
LAST_RESULT = None

# ACT Silu/Gelu tables are broken on this stack (wrong values or exec-unit
# crash); always emit sigmoid + x*sig(x) on DVE.
CFG = {"silu": bool(int(os.environ.get("KSILU", "0")))}


def _build(cfg=None):
    cfg = {**CFG, **(cfg or {})}
    use_silu = cfg["silu"]
    nc = bacc.Bacc(
        "TRN2", target_bir_lowering=False, debug=False, num_devices=NCORES
    )

    def din(name, shape, dt):
        return nc.dram_tensor(name, list(shape), dt, kind="ExternalInput").ap()

    ht8_d = din("ht8", [P, KD, SEQ], F8)     # h pre-transposed, fp8
    wi8_d = din("wi8", [P, KD, NCOL], F8)    # 16*Wi, qk cols permuted
    wo8_d = din("wo8", [P, UT, DIM], F8)     # 16*Wo
    hres_d = din("hres", [SLAB, DIM], F32)   # own-slab h + bo
    # output in bf16 (residual dominates; host casts back to f32)
    cc_d = din("cc", [P, SEQ], BF16)
    ss_d = din("ss", [P, SEQ], BF16)
    gbb_d = din("gbb", [P, 5], F32)          # qg*c, qb*c, kg, kb, b_qk
    bu_d = din("bu", [P, UT], F32)           # bi_u per-partition
    bv8_d = din("bv8", [1, UV], F8)          # 16*bi_v
    out_d = nc.dram_tensor("out", [SLAB, DIM], BF16, kind="ExternalOutput").ap()

    def silu_act(out, in_, bias=0.0, scale=1.0, pool=None, shape=None):
        """silu from PSUM: single ACT op if the table is available, else
        sigmoid on ACT + x*sig on DVE (CoreSim fallback)."""
        if use_silu:
            nc.scalar.activation(
                out=out, in_=in_, func=AF.Silu, bias=bias, scale=scale
            )
        else:
            n = shape[-1] * (shape[1] if len(shape) > 2 else 1)
            sg = pool.tile(list(shape), BF16, tag=f"sg{n}", name="sg", bufs=2)
            nc.scalar.activation(
                out=sg, in_=in_, func=AF.Sigmoid, bias=bias, scale=scale
            )
            if isinstance(bias, float) and bias == 0.0:
                nc.vector.scalar_tensor_tensor(
                    out=out, in0=in_, scalar=scale, in1=sg,
                    op0=OP.mult, op1=OP.mult,
                )
            else:
                xx = pool.tile(list(shape), BF16, tag=f"xx{n}", name="xx",
                               bufs=2)
                nc.vector.tensor_scalar(
                    out=xx, in0=in_, scalar1=scale, scalar2=bias,
                    op0=OP.mult, op1=OP.add,
                )
                nc.vector.tensor_mul(out=out, in0=xx, in1=sg)

    with tile.TileContext(nc) as tc:
        with (
            tc.tile_pool(name="consts", bufs=1) as consts,
            tc.tile_pool(name="persist", bufs=1) as persist,
            tc.tile_pool(name="work", bufs=2) as work,
        ):
            gbb_sb = consts.tile([P, 5], F32, tag="gbb", name="gbb_sb")
            bu_sb = consts.tile([P, UT], F32, tag="bu", name="bu_sb")
            bv8_sb = consts.tile([1, UV], F8, tag="bv8", name="bv8_sb")
            ones8_sb = consts.tile([1, P], F8, tag="ones8", name="ones8_sb")
            eps_sb = consts.tile([P, 1], F32, tag="eps", name="eps_sb")
            nc.sync.dma_start(out=gbb_sb, in_=gbb_d)
            nc.sync.dma_start(out=bu_sb, in_=bu_d)
            nc.sync.dma_start(out=bv8_sb, in_=bv8_d)
            nc.vector.memset(ones8_sb, 1.0)
            nc.vector.memset(eps_sb, EPS)

            p1_cm = tc.tile_pool(name="p1", bufs=1)
            p1 = p1_cm.__enter__()
            ht8 = p1.tile([P, KD, SEQ], F8, tag="ht8", name="ht8")
            wi8 = p1.tile([P, KD, NCOL], F8, tag="wi8", name="wi8")
            cc_sb = p1.tile([P, SEQ], BF16, tag="cc", name="cc_sb")
            ss_sb = p1.tile([P, SEQ], BF16, tag="ss", name="ss_sb")
            nc.sync.dma_start(out=ht8, in_=ht8_d)
            nc.sync.dma_start(out=wi8, in_=wi8_d)
            nc.sync.dma_start(out=cc_sb, in_=cc_d)
            nc.sync.dma_start(out=ss_sb, in_=ss_d)

            v8 = persist.tile([P, KT, UV], F8, tag="v8", name="v8")
            kT_sb = persist.tile([P, SEQ], BF16, tag="kT", name="kT_sb")
            qT_sb = persist.tile([P, SLAB], BF16, tag="qT", name="qT_sb")
            u8 = persist.tile([P, UT, SLAB], F8, tag="u8", name="u8")
            wo8 = persist.tile([P, UT, DIM], F8, tag="wo8", name="wo8")
            nc.sync.dma_start(out=wo8, in_=wo8_d)

            # ---- 1a: qk feature-major + gamma/beta + rope -> kT, qT ----
            with tc.tile_pool(name="pq", bufs=2, space="PSUM") as pqp:
                for chp in range(NCH // 2):
                    pq = pqp.tile([P, 2, CH], F32, tag="pq", name="pq")
                    for kd2 in range(KD2):
                        for chl in range(2):
                            nc.tensor.matmul(
                                pq[:, chl, :],
                                wi8[:, 2 * kd2:2 * kd2 + 2, 2 * UV:NCOL],
                                ht8[:, 2 * kd2:2 * kd2 + 2,
                                    (2 * chp + chl) * CH:(2 * chp + chl + 1) * CH],
                                start=(kd2 == 0), stop=(kd2 == KD2 - 1),
                                perf_mode=DR,
                            )
                    for chl in range(2):
                        ch = 2 * chp + chl
                        t0 = ch * CH
                        qk_f = work.tile([P, CH], BF16, tag="qkf", name="qk_f",
                                         bufs=2)
                        silu_act(qk_f, pq[:, chl, :], bias=gbb_sb[:, 4:5],
                                 scale=1.0 / 16, pool=work, shape=[P, CH])
                        targets = [(kT_sb[:, t0:t0 + CH], 2)]
                        if ch < OWN_CH:
                            targets.append((qT_sb[:, t0:t0 + CH], 0))
                        for dst, gi in targets:
                            pre = work.tile([P, CH], BF16, tag="pre",
                                            name="pre", bufs=2)
                            nc.vector.tensor_scalar(
                                out=pre, in0=qk_f,
                                scalar1=gbb_sb[:, gi:gi + 1],
                                scalar2=gbb_sb[:, gi + 1:gi + 2],
                                op0=OP.mult, op1=OP.add,
                            )
                            x1 = pre[0:HALF, :]
                            x2 = pre[HALF:P, :]
                            ta = work.tile([HALF, CH], BF16, tag="ta",
                                           name="ta", bufs=2)
                            tb = work.tile([HALF, CH], BF16, tag="tb",
                                           name="tb", bufs=2)
                            nc.vector.tensor_mul(
                                out=ta, in0=x1, in1=cc_sb[0:HALF, t0:t0 + CH]
                            )
                            nc.vector.tensor_mul(
                                out=tb, in0=x2, in1=ss_sb[HALF:P, t0:t0 + CH]
                            )
                            nc.vector.tensor_sub(
                                out=dst[0:HALF, :], in0=ta, in1=tb
                            )
                            tc_ = work.tile([HALF, CH], BF16, tag="ta",
                                            name="tc_", bufs=2)
                            td = work.tile([HALF, CH], BF16, tag="tb",
                                           name="td", bufs=2)
                            nc.vector.tensor_mul(
                                out=tc_, in0=x1, in1=ss_sb[0:HALF, t0:t0 + CH]
                            )
                            nc.vector.tensor_mul(
                                out=td, in0=x2, in1=cc_sb[HALF:P, t0:t0 + CH]
                            )
                            nc.vector.tensor_add(
                                out=dst[HALF:P, :], in0=tc_, in1=td
                            )

            # ---- 1b: v token-major, full sequence ----
            with tc.tile_pool(name="pv", bufs=2, space="PSUM") as pvp:
                for tt in range(KT):
                    pv = pvp.tile([P, VT, CH], F32, tag="pv", name="pv")
                    for vc in range(VT):
                        nc.tensor.matmul(
                            pv[:, vc, :], ones8_sb,
                            bv8_sb[:, vc * CH:(vc + 1) * CH],
                            start=True, stop=False,
                        )
                    for kd2 in range(KD2):
                        for vc in range(VT):
                            nc.tensor.matmul(
                                pv[:, vc, :],
                                ht8[:, 2 * kd2:2 * kd2 + 2,
                                    tt * P:(tt + 1) * P],
                                wi8[:, 2 * kd2:2 * kd2 + 2,
                                    UV + vc * CH:UV + (vc + 1) * CH],
                                start=False, stop=(kd2 == KD2 - 1),
                                perf_mode=DR,
                            )
                    silu_act(v8[:, tt, :], pv[:, :, :], scale=1.0 / 16,
                             pool=work, shape=[P, UV])

            # ---- 1c: u feature-major for own tokens ----
            with tc.tile_pool(name="pu", bufs=2, space="PSUM") as pup:
                for ut in range(UT):
                    pu = pup.tile([P, OWN_CH, CH], F32, tag="pu", name="pu")
                    for kd2 in range(KD2):
                        for qc in range(OWN_CH):
                            nc.tensor.matmul(
                                pu[:, qc, :],
                                wi8[:, 2 * kd2:2 * kd2 + 2,
                                    ut * P:(ut + 1) * P],
                                ht8[:, 2 * kd2:2 * kd2 + 2,
                                    qc * CH:(qc + 1) * CH],
                                start=(kd2 == 0), stop=(kd2 == KD2 - 1),
                                perf_mode=DR,
                            )
                    silu_act(u8[:, ut, :], pu[:, :, :],
                             bias=bu_sb[:, ut:ut + 1], scale=1.0 / 16,
                             pool=work, shape=[P, SLAB])

            # ---- 2: attention + output, two query-pair phases ----
            p1_cm.__exit__(None, None, None)
            with (
                tc.tile_pool(name="p2", bufs=1) as p2,
                tc.tile_pool(name="ps_s", bufs=2, space="PSUM") as ps_s,
                tc.tile_pool(name="ps_av", bufs=2, space="PSUM") as ps_av,
                tc.tile_pool(name="ps_o", bufs=1, space="PSUM") as ps_o,
            ):
                at8 = p2.tile([P, KT, QPW], F8, tag="at8", name="at8")
                g8 = p2.tile([P, UT, QPW], F8, tag="g8", name="g8")
                for qph in range(QPH):
                    qg0 = qph * QPW
                    # scores + relu^2 (kT-stationary shared by both chunks)
                    for kt in range(KT):
                        for qcl in range(2):
                            ps = ps_s.tile([P, CH], F32, tag="ps", name="ps")
                            nc.tensor.matmul(
                                ps, kT_sb[:, kt * P:(kt + 1) * P],
                                qT_sb[:, qg0 + qcl * CH:qg0 + (qcl + 1) * CH],
                                start=True, stop=True,
                            )
                            rl = work.tile([P, CH], BF16, tag="rl", name="rl",
                                           bufs=3)
                            nc.scalar.activation(out=rl, in_=ps, func=AF.Relu)
                            nc.vector.tensor_mul(
                                out=at8[:, kt, qcl * CH:(qcl + 1) * CH],
                                in0=rl, in1=rl,
                            )
                    # Av accumulation (v-stationary shared by both chunks)
                    for ut in range(UT):
                        pav = ps_av.tile([P, 2, CH], F32, tag="pav",
                                         name="pav")
                        for kt2 in range(KT2):
                            for qcl in range(2):
                                nc.tensor.matmul(
                                    pav[:, qcl, :],
                                    v8[:, 2 * kt2:2 * kt2 + 2,
                                       ut * P:(ut + 1) * P],
                                    at8[:, 2 * kt2:2 * kt2 + 2,
                                        qcl * CH:(qcl + 1) * CH],
                                    start=(kt2 == 0), stop=(kt2 == KT2 - 1),
                                    perf_mode=DR,
                                )
                        for qcl in range(2):
                            nc.vector.scalar_tensor_tensor(
                                out=g8[:, ut, qcl * CH:(qcl + 1) * CH],
                                in0=pav[:, qcl, :], scalar=1.0,
                                in1=u8[:, ut, qg0 + qcl * CH:
                                       qg0 + (qcl + 1) * CH],
                                op0=OP.mult, op1=OP.mult,
                            )
                    # output projection + residual + RMS norm
                    for tl in range(QPW // P):
                        tok_l = tl * P
                        tok_g = qg0 + tok_l
                        po_a = ps_o.tile([P, CH], F32, tag="poa", name="po_a")
                        po_b = ps_o.tile([P, DIM - CH], F32, tag="pob",
                                         name="po_b")
                        for u2 in range(UT2):
                            g_t = g8[:, 2 * u2:2 * u2 + 2, tok_l:tok_l + P]
                            nc.tensor.matmul(
                                po_a, g_t, wo8[:, 2 * u2:2 * u2 + 2, 0:CH],
                                start=(u2 == 0), stop=(u2 == UT2 - 1),
                                perf_mode=DR,
                            )
                            nc.tensor.matmul(
                                po_b, g_t, wo8[:, 2 * u2:2 * u2 + 2, CH:DIM],
                                start=(u2 == 0), stop=(u2 == UT2 - 1),
                                perf_mode=DR,
                            )
                        hres = work.tile([P, DIM], F32, tag="hres",
                                         name="hres", bufs=2)
                        nc.sync.dma_start(
                            out=hres, in_=hres_d[tok_g:tok_g + P, :]
                        )
                        o_sb = work.tile([P, DIM], F32, tag="osb",
                                         name="o_sb", bufs=2)
                        nc.vector.scalar_tensor_tensor(
                            out=o_sb[:, 0:CH], in0=po_a, scalar=OSC,
                            in1=hres[:, 0:CH], op0=OP.mult, op1=OP.add,
                        )
                        nc.vector.scalar_tensor_tensor(
                            out=o_sb[:, CH:DIM], in0=po_b, scalar=OSC,
                            in1=hres[:, CH:DIM], op0=OP.mult, op1=OP.add,
                        )
                        o2 = work.tile([P, DIM], F8, tag="o2", name="o2",
                                       bufs=2)
                        ms = work.tile([P, 1], F32, tag="ms", name="ms")
                        nc.scalar.activation(
                            out=o2, in_=o_sb, func=AF.Square, accum_out=ms
                        )
                        sd = work.tile([P, 1], F32, tag="sd", name="sd")
                        nc.scalar.activation(
                            out=sd, in_=ms, func=AF.Sqrt,
                            bias=eps_sb[:, 0:1], scale=1.0 / DIM,
                        )
                        rinv = work.tile([P, 1], F32, tag="rinv", name="rinv")
                        nc.vector.reciprocal(out=rinv, in_=sd)
                        ofin = work.tile([P, DIM], BF16, tag="ofin",
                                         name="ofin", bufs=2)
                        nc.vector.tensor_scalar_mul(ofin, o_sb, rinv[:, 0:1])
                        nc.sync.dma_start(
                            out=out_d[tok_g:tok_g + P, :], in_=ofin
                        )
    nc.compile()
    return nc


def _get_nc(cfg=None):
    key = ("nc", tuple(sorted((cfg or CFG).items())))
    if key not in _cache:
        _cache[key] = _build(cfg)
    return _cache[key]


def _host_prep(hidden_states, Wi, bi, Wo, bo, q_gamma, q_beta, k_gamma, k_beta):
    h = np.ascontiguousarray(np.asarray(hidden_states, dtype=np.float32))
    Wi = np.asarray(Wi, dtype=np.float32)
    bi = np.asarray(bi, dtype=np.float32)
    Wo = np.asarray(Wo, dtype=np.float32)
    bo = np.asarray(bo, dtype=np.float32)

    perm = np.concatenate([np.arange(0, KEY, 2), np.arange(1, KEY, 2)])
    Wi_p = Wi.copy()
    Wi_p[:, 2 * UV:] = Wi_p[:, 2 * UV:][:, perm]
    # scale into e4m3 normal range; 1/16 applied after psum
    wi8 = np.ascontiguousarray(
        (16.0 * Wi_p).reshape(KD, P, NCOL).transpose(1, 0, 2)
    ).astype(ml_dtypes.float8_e4m3)
    wo8 = np.ascontiguousarray(
        (16.0 * Wo).reshape(UT, P, DIM).transpose(1, 0, 2)
    ).astype(ml_dtypes.float8_e4m3)

    c = float(KEY ** -0.5)
    gbb = np.stack(
        [
            np.asarray(q_gamma, np.float32)[perm] * c,
            np.asarray(q_beta, np.float32)[perm] * c,
            np.asarray(k_gamma, np.float32)[perm],
            np.asarray(k_beta, np.float32)[perm],
            bi[2 * UV:][perm],
        ],
        axis=1,
    ).astype(np.float32)
    bu = np.ascontiguousarray(
        bi[:UV].reshape(UT, P).T
    ).astype(np.float32)
    bv8 = (16.0 * bi[UV:2 * UV]).reshape(1, UV).astype(ml_dtypes.float8_e4m3)

    omega = 1.0 / (10000.0 ** (np.arange(HALF, dtype=np.float32) / HALF))
    ang = np.arange(SEQ, dtype=np.float32)[:, None] * omega[None, :]
    cos_t = np.cos(ang).T
    sin_t = np.sin(ang).T
    cc_full = np.concatenate([cos_t, cos_t], axis=0).astype(ml_dtypes.bfloat16)
    ss_full = np.concatenate([sin_t, sin_t], axis=0).astype(ml_dtypes.bfloat16)

    shared = {"wi8": wi8, "wo8": wo8, "gbb": gbb, "bu": bu, "bv8": bv8}
    in_maps = []
    for core in range(NCORES):
        b, s = divmod(core, 2)
        order = np.concatenate(
            [
                np.arange(s * SLAB, (s + 1) * SLAB),
                np.arange((1 - s) * SLAB, (2 - s) * SLAB),
            ]
        )
        hb = h[b][order]
        m = dict(shared)
        m["ht8"] = np.ascontiguousarray(
            hb.T.reshape(KD, P, SEQ).transpose(1, 0, 2)
        ).astype(ml_dtypes.float8_e4m3)
        m["hres"] = np.ascontiguousarray(hb[:SLAB] + bo[None, :])
        m["cc"] = np.ascontiguousarray(cc_full[:, order])
        m["ss"] = np.ascontiguousarray(ss_full[:, order])
        in_maps.append(m)
    return in_maps


def kernel(hidden_states, Wi, bi, Wo, bo, q_gamma, q_beta, k_gamma, k_beta):
    global LAST_RESULT
    nc = _get_nc()
    in_maps = _host_prep(
        hidden_states, Wi, bi, Wo, bo, q_gamma, q_beta, k_gamma, k_beta
    )
    res = bass_utils.run_bass_kernel_spmd(
        nc,
        in_maps,
        core_ids=list(range(NCORES)),
        trace=bool(int(os.environ.get("KTRACE", "0"))),
    )
    LAST_RESULT = res
    out = np.empty((NB, SEQ, DIM), dtype=np.float32)
    for core in range(NCORES):
        b, s = divmod(core, 2)
        out[b, s * SLAB:(s + 1) * SLAB] = res.results[core]["out"].astype(
            np.float32
        )
    return out


# revision 25
# speedup vs baseline: 1.4523x; 1.0038x over previous
"""GAU (Gated Attention Unit) layer kernel for Trainium2, 8 NeuronCores.

Sharding: query-sequence-parallel within batch. 4 batches x 2 query slabs
of 2048 -> 8 cores. Each core receives the full 4096-token sequence of its
batch (rows reordered so its own query slab comes first), computes the
full-sequence K/V projection, and attention + output projection for its
own 2048 queries.

v2: all heavy lifting pre-staged on host (h pre-transposed + cast fp8,
Wi/Wo pre-cast fp8 with x16 scale), silu on the ACT engine (the silu
table exists on TRN2 even though CoreSim lacks it), per-partition biases
via the ACT bias path, fp8 DoubleRow output projection, bo folded into
the residual h on host. Per-core dataflow (fp32 PSUM accumulation):
  1a. qk = silu(h@Wi_qk + b) feature-major; gamma/beta + RoPE -> qT,kT
      (qk columns host-permuted evens-first; 1/sqrt(d) folded into
      q_gamma/q_beta host-side)
  1b. v = silu(h@Wi_v + b) token-major [tok,1536] fp8 (bias via ones
      matmul into PSUM; silu in one ACT op per 128-token row)
  1c. u = silu(h@Wi_u + b) feature-major fp8, bias via ACT bias
  2.  two query-pair phases (qph x 1024 tokens): scores^T = kT.T@qT,
      at = relu(s)^2 (ACT relu + DVE square, fp8), Av^T accumulated
      over 32 key tiles fp8-DR, g = u * Av^T fp8, out = g@Wo fp8-DR,
      o = po/65536 + (h+bo), RMS-normalize, DMA out.
"""

import os

import ml_dtypes
import numpy as np

import concourse.bass as bass
import concourse.mybir as mybir
import concourse.tile as tile
from concourse import bacc, bass_utils

P = 128
SEQ = 4096
DIM = 768
NCOL = 3200
UV = 1536
KEY = 128
HALF = 64
SLAB = 2048
KD = DIM // P        # 6 feature k-tiles
KD2 = KD // 2        # 3 DoubleRow feature pairs
KT = SEQ // P        # 32 key-token tiles
KT2 = KT // 2        # 16 DoubleRow key pairs
CH = 512
NCH = SEQ // CH      # 8 token chunks
OWN_CH = SLAB // CH  # 4 own (query) chunks
VT = UV // CH        # 3 v-column chunks
UT = UV // P         # 12 u/v feature tiles
UT2 = UT // 2        # 6 DoubleRow u pairs
NB = 4
NCORES = 8
EPS = 1e-12
QPH = 2              # query-pair phases
QPW = SLAB // QPH    # 1024 tokens per phase
OSC = 1.0 / (16.0 * SEQ)  # output descale: wo x16, at carries xSEQ

F32 = mybir.dt.float32
BF16 = mybir.dt.bfloat16
F8 = mybir.dt.float8e4
OP = mybir.AluOpType
AF = mybir.ActivationFunctionType
DR = mybir.MatmulPerfMode.DoubleRow

_cache = {}
LAST_RESULT = None

# ACT Silu/Gelu tables are broken on this stack (wrong values or exec-unit
# crash); always emit sigmoid + x*sig(x) on DVE.
CFG = {"silu": bool(int(os.environ.get("KSILU", "0")))}


def _build(cfg=None):
    cfg = {**CFG, **(cfg or {})}
    use_silu = cfg["silu"]
    nc = bacc.Bacc(
        "TRN2", target_bir_lowering=False, debug=False, num_devices=NCORES
    )

    def din(name, shape, dt):
        return nc.dram_tensor(name, list(shape), dt, kind="ExternalInput").ap()

    ht8_d = din("ht8", [P, KD, SEQ], F8)     # h pre-transposed, fp8
    wi8v_d = din("wi8v", [P, KD, UV], F8)    # 16*Wi v block
    wi8u_d = din("wi8u", [P, KD, UV], F8)    # 16*Wi u block
    wi8qk_d = din("wi8qk", [P, KD, KEY], F8)  # 16*Wi qk block, permuted
    wo8_d = din("wo8", [P, UT, DIM], F8)     # 16*Wo
    hres_d = din("hres", [SLAB, DIM], F32)   # own-slab h + bo
    # output in bf16 (residual dominates; host casts back to f32)
    cc_d = din("cc", [P, SEQ], BF16)
    ss_d = din("ss", [P, SEQ], BF16)
    gbb_d = din("gbb", [P, 5], F32)          # qg*c, qb*c, kg, kb, b_qk
    bu_d = din("bu", [P, UT], F32)           # bi_u per-partition
    bv8_d = din("bv8", [1, UV], F8)          # 16*bi_v
    out_d = nc.dram_tensor("out", [SLAB, DIM], BF16, kind="ExternalOutput").ap()

    def silu_act(out, in_, bias=0.0, scale=1.0, pool=None, shape=None):
        """silu from PSUM: single ACT op if the table is available, else
        sigmoid on ACT + x*sig on DVE (CoreSim fallback)."""
        if use_silu:
            nc.scalar.activation(
                out=out, in_=in_, func=AF.Silu, bias=bias, scale=scale
            )
        else:
            n = shape[-1] * (shape[1] if len(shape) > 2 else 1)
            sg = pool.tile(list(shape), BF16, tag=f"sg{n}", name="sg", bufs=2)
            nc.scalar.activation(
                out=sg, in_=in_, func=AF.Sigmoid, bias=bias, scale=scale
            )
            if isinstance(bias, float) and bias == 0.0:
                nc.vector.scalar_tensor_tensor(
                    out=out, in0=in_, scalar=scale, in1=sg,
                    op0=OP.mult, op1=OP.mult,
                )
            else:
                xx = pool.tile(list(shape), BF16, tag=f"xx{n}", name="xx",
                               bufs=2)
                nc.vector.tensor_scalar(
                    out=xx, in0=in_, scalar1=scale, scalar2=bias,
                    op0=OP.mult, op1=OP.add,
                )
                nc.vector.tensor_mul(out=out, in0=xx, in1=sg)

    with tile.TileContext(nc) as tc:
        with (
            tc.tile_pool(name="consts", bufs=1) as consts,
            tc.tile_pool(name="persist", bufs=1) as persist,
            tc.tile_pool(name="work", bufs=2) as work,
        ):
            gbb_sb = consts.tile([P, 5], F32, tag="gbb", name="gbb_sb")
            bu_sb = consts.tile([P, UT], F32, tag="bu", name="bu_sb")
            bv8_sb = consts.tile([1, UV], F8, tag="bv8", name="bv8_sb")
            ones8_sb = consts.tile([1, P], F8, tag="ones8", name="ones8_sb")
            eps_sb = consts.tile([P, 1], F32, tag="eps", name="eps_sb")
            nc.sync.dma_start(out=gbb_sb, in_=gbb_d)
            nc.sync.dma_start(out=bu_sb, in_=bu_d)
            nc.sync.dma_start(out=bv8_sb, in_=bv8_d)
            nc.vector.memset(ones8_sb, 1.0)
            nc.vector.memset(eps_sb, EPS)

            p1_cm = tc.tile_pool(name="p1", bufs=1)
            p1 = p1_cm.__enter__()
            ht8 = p1.tile([P, KD, SEQ], F8, tag="ht8", name="ht8")
            wi8v = p1.tile([P, KD, UV], F8, tag="wi8v", name="wi8v")
            wi8u = p1.tile([P, KD, UV], F8, tag="wi8u", name="wi8u")
            wi8qk = p1.tile([P, KD, KEY], F8, tag="wi8qk", name="wi8qk")
            cc_sb = p1.tile([P, SEQ], BF16, tag="cc", name="cc_sb")
            ss_sb = p1.tile([P, SEQ], BF16, tag="ss", name="ss_sb")
            # DMA order matters: v's operands first so PE can start early
            nc.sync.dma_start(out=ht8, in_=ht8_d)
            nc.sync.dma_start(out=wi8v, in_=wi8v_d)
            nc.sync.dma_start(out=wi8qk, in_=wi8qk_d)
            nc.sync.dma_start(out=cc_sb, in_=cc_d)
            nc.sync.dma_start(out=ss_sb, in_=ss_d)
            nc.sync.dma_start(out=wi8u, in_=wi8u_d)

            v8 = persist.tile([P, KT, UV], F8, tag="v8", name="v8")
            kT_sb = persist.tile([P, SEQ], BF16, tag="kT", name="kT_sb")
            qT_sb = persist.tile([P, SLAB], BF16, tag="qT", name="qT_sb")
            u8 = persist.tile([P, UT, SLAB], F8, tag="u8", name="u8")
            wo8 = persist.tile([P, UT, DIM], F8, tag="wo8", name="wo8")
            nc.sync.dma_start(out=wo8, in_=wo8_d)

            # ---- 1a: v token-major, full sequence ----
            with tc.tile_pool(name="pv", bufs=2, space="PSUM") as pvp:
                for tt in range(KT):
                    pv = pvp.tile([P, VT, CH], F32, tag="pv", name="pv")
                    for vc in range(VT):
                        nc.tensor.matmul(
                            pv[:, vc, :], ones8_sb,
                            bv8_sb[:, vc * CH:(vc + 1) * CH],
                            start=True, stop=False,
                        )
                    for kd2 in range(KD2):
                        for vc in range(VT):
                            nc.tensor.matmul(
                                pv[:, vc, :],
                                ht8[:, 2 * kd2:2 * kd2 + 2,
                                    tt * P:(tt + 1) * P],
                                wi8v[:, 2 * kd2:2 * kd2 + 2,
                                     vc * CH:(vc + 1) * CH],
                                start=False, stop=(kd2 == KD2 - 1),
                                perf_mode=DR,
                            )
                    silu_act(v8[:, tt, :], pv[:, :, :], scale=1.0 / 16,
                             pool=work, shape=[P, UV])

            # ---- 1b: qk feature-major + gamma/beta + rope -> kT, qT ----
            with tc.tile_pool(name="pq", bufs=2, space="PSUM") as pqp:
                for chp in range(NCH // 2):
                    pq = pqp.tile([P, 2, CH], F32, tag="pq", name="pq")
                    for kd2 in range(KD2):
                        for chl in range(2):
                            nc.tensor.matmul(
                                pq[:, chl, :],
                                wi8qk[:, 2 * kd2:2 * kd2 + 2, :],
                                ht8[:, 2 * kd2:2 * kd2 + 2,
                                    (2 * chp + chl) * CH:(2 * chp + chl + 1) * CH],
                                start=(kd2 == 0), stop=(kd2 == KD2 - 1),
                                perf_mode=DR,
                            )
                    for chl in range(2):
                        ch = 2 * chp + chl
                        t0 = ch * CH
                        qk_f = work.tile([P, CH], BF16, tag="qkf", name="qk_f",
                                         bufs=2)
                        silu_act(qk_f, pq[:, chl, :], bias=gbb_sb[:, 4:5],
                                 scale=1.0 / 16, pool=work, shape=[P, CH])
                        targets = [(kT_sb[:, t0:t0 + CH], 2)]
                        if ch < OWN_CH:
                            targets.append((qT_sb[:, t0:t0 + CH], 0))
                        for dst, gi in targets:
                            pre = work.tile([P, CH], BF16, tag="pre",
                                            name="pre", bufs=2)
                            nc.vector.tensor_scalar(
                                out=pre, in0=qk_f,
                                scalar1=gbb_sb[:, gi:gi + 1],
                                scalar2=gbb_sb[:, gi + 1:gi + 2],
                                op0=OP.mult, op1=OP.add,
                            )
                            x1 = pre[0:HALF, :]
                            x2 = pre[HALF:P, :]
                            ta = work.tile([HALF, CH], BF16, tag="ta",
                                           name="ta", bufs=2)
                            tb = work.tile([HALF, CH], BF16, tag="tb",
                                           name="tb", bufs=2)
                            nc.vector.tensor_mul(
                                out=ta, in0=x1, in1=cc_sb[0:HALF, t0:t0 + CH]
                            )
                            nc.vector.tensor_mul(
                                out=tb, in0=x2, in1=ss_sb[HALF:P, t0:t0 + CH]
                            )
                            nc.vector.tensor_sub(
                                out=dst[0:HALF, :], in0=ta, in1=tb
                            )
                            tc_ = work.tile([HALF, CH], BF16, tag="ta",
                                            name="tc_", bufs=2)
                            td = work.tile([HALF, CH], BF16, tag="tb",
                                           name="td", bufs=2)
                            nc.vector.tensor_mul(
                                out=tc_, in0=x1, in1=ss_sb[0:HALF, t0:t0 + CH]
                            )
                            nc.vector.tensor_mul(
                                out=td, in0=x2, in1=cc_sb[HALF:P, t0:t0 + CH]
                            )
                            nc.vector.tensor_add(
                                out=dst[HALF:P, :], in0=tc_, in1=td
                            )

            # ---- 1c: u feature-major for own tokens ----
            with tc.tile_pool(name="pu", bufs=2, space="PSUM") as pup:
                for ut in range(UT):
                    pu = pup.tile([P, OWN_CH, CH], F32, tag="pu", name="pu")
                    for kd2 in range(KD2):
                        for qc in range(OWN_CH):
                            nc.tensor.matmul(
                                pu[:, qc, :],
                                wi8u[:, 2 * kd2:2 * kd2 + 2,
                                     ut * P:(ut + 1) * P],
                                ht8[:, 2 * kd2:2 * kd2 + 2,
                                    qc * CH:(qc + 1) * CH],
                                start=(kd2 == 0), stop=(kd2 == KD2 - 1),
                                perf_mode=DR,
                            )
                    silu_act(u8[:, ut, :], pu[:, :, :],
                             bias=bu_sb[:, ut:ut + 1], scale=1.0 / 16,
                             pool=work, shape=[P, SLAB])

            # ---- 2: attention + output, two query-pair phases ----
            p1_cm.__exit__(None, None, None)
            with (
                tc.tile_pool(name="p2", bufs=1) as p2,
                tc.tile_pool(name="ps_s", bufs=2, space="PSUM") as ps_s,
                tc.tile_pool(name="ps_av", bufs=2, space="PSUM") as ps_av,
                tc.tile_pool(name="ps_o", bufs=1, space="PSUM") as ps_o,
            ):
                at8 = p2.tile([P, KT, QPW], F8, tag="at8", name="at8")
                g8 = p2.tile([P, UT, QPW], F8, tag="g8", name="g8")
                for qph in range(QPH):
                    qg0 = qph * QPW
                    # scores + relu^2 (kT-stationary shared by both chunks)
                    for kt in range(KT):
                        for qcl in range(2):
                            ps = ps_s.tile([P, CH], F32, tag="ps", name="ps")
                            nc.tensor.matmul(
                                ps, kT_sb[:, kt * P:(kt + 1) * P],
                                qT_sb[:, qg0 + qcl * CH:qg0 + (qcl + 1) * CH],
                                start=True, stop=True,
                            )
                            rl = work.tile([P, CH], BF16, tag="rl", name="rl",
                                           bufs=3)
                            nc.scalar.activation(out=rl, in_=ps, func=AF.Relu)
                            nc.vector.tensor_mul(
                                out=at8[:, kt, qcl * CH:(qcl + 1) * CH],
                                in0=rl, in1=rl,
                            )
                    # Av accumulation (v-stationary shared by both chunks)
                    for ut in range(UT):
                        pav = ps_av.tile([P, 2, CH], F32, tag="pav",
                                         name="pav")
                        for kt2 in range(KT2):
                            for qcl in range(2):
                                nc.tensor.matmul(
                                    pav[:, qcl, :],
                                    v8[:, 2 * kt2:2 * kt2 + 2,
                                       ut * P:(ut + 1) * P],
                                    at8[:, 2 * kt2:2 * kt2 + 2,
                                        qcl * CH:(qcl + 1) * CH],
                                    start=(kt2 == 0), stop=(kt2 == KT2 - 1),
                                    perf_mode=DR,
                                )
                        for qcl in range(2):
                            nc.vector.scalar_tensor_tensor(
                                out=g8[:, ut, qcl * CH:(qcl + 1) * CH],
                                in0=pav[:, qcl, :], scalar=1.0,
                                in1=u8[:, ut, qg0 + qcl * CH:
                                       qg0 + (qcl + 1) * CH],
                                op0=OP.mult, op1=OP.mult,
                            )
                    # output projection + residual + RMS norm
                    for tl in range(QPW // P):
                        tok_l = tl * P
                        tok_g = qg0 + tok_l
                        po_a = ps_o.tile([P, CH], F32, tag="poa", name="po_a")
                        po_b = ps_o.tile([P, DIM - CH], F32, tag="pob",
                                         name="po_b")
                        for u2 in range(UT2):
                            g_t = g8[:, 2 * u2:2 * u2 + 2, tok_l:tok_l + P]
                            nc.tensor.matmul(
                                po_a, g_t, wo8[:, 2 * u2:2 * u2 + 2, 0:CH],
                                start=(u2 == 0), stop=(u2 == UT2 - 1),
                                perf_mode=DR,
                            )
                            nc.tensor.matmul(
                                po_b, g_t, wo8[:, 2 * u2:2 * u2 + 2, CH:DIM],
                                start=(u2 == 0), stop=(u2 == UT2 - 1),
                                perf_mode=DR,
                            )
                        hres = work.tile([P, DIM], F32, tag="hres",
                                         name="hres", bufs=2)
                        nc.sync.dma_start(
                            out=hres, in_=hres_d[tok_g:tok_g + P, :]
                        )
                        o_sb = work.tile([P, DIM], F32, tag="osb",
                                         name="o_sb", bufs=2)
                        nc.vector.scalar_tensor_tensor(
                            out=o_sb[:, 0:CH], in0=po_a, scalar=OSC,
                            in1=hres[:, 0:CH], op0=OP.mult, op1=OP.add,
                        )
                        nc.vector.scalar_tensor_tensor(
                            out=o_sb[:, CH:DIM], in0=po_b, scalar=OSC,
                            in1=hres[:, CH:DIM], op0=OP.mult, op1=OP.add,
                        )
                        o2 = work.tile([P, DIM], F8, tag="o2", name="o2",
                                       bufs=2)
                        ms = work.tile([P, 1], F32, tag="ms", name="ms")
                        nc.scalar.activation(
                            out=o2, in_=o_sb, func=AF.Square, accum_out=ms
                        )
                        sd = work.tile([P, 1], F32, tag="sd", name="sd")
                        nc.scalar.activation(
                            out=sd, in_=ms, func=AF.Sqrt,
                            bias=eps_sb[:, 0:1], scale=1.0 / DIM,
                        )
                        rinv = work.tile([P, 1], F32, tag="rinv", name="rinv")
                        nc.vector.reciprocal(out=rinv, in_=sd)
                        ofin = work.tile([P, DIM], BF16, tag="ofin",
                                         name="ofin", bufs=2)
                        nc.scalar.mul(ofin, o_sb, rinv[:, 0:1])
                        nc.sync.dma_start(
                            out=out_d[tok_g:tok_g + P, :], in_=ofin
                        )
    nc.compile()
    return nc


def _get_nc(cfg=None):
    key = ("nc", tuple(sorted((cfg or CFG).items())))
    if key not in _cache:
        _cache[key] = _build(cfg)
    return _cache[key]


def _host_prep(hidden_states, Wi, bi, Wo, bo, q_gamma, q_beta, k_gamma, k_beta):
    h = np.ascontiguousarray(np.asarray(hidden_states, dtype=np.float32))
    Wi = np.asarray(Wi, dtype=np.float32)
    bi = np.asarray(bi, dtype=np.float32)
    Wo = np.asarray(Wo, dtype=np.float32)
    bo = np.asarray(bo, dtype=np.float32)

    perm = np.concatenate([np.arange(0, KEY, 2), np.arange(1, KEY, 2)])
    # scale into e4m3 normal range; 1/16 applied after psum
    wi8 = np.ascontiguousarray(
        (16.0 * Wi).reshape(KD, P, NCOL).transpose(1, 0, 2)
    ).astype(ml_dtypes.float8_e4m3)
    wi8u = np.ascontiguousarray(wi8[:, :, :UV])
    wi8v = np.ascontiguousarray(wi8[:, :, UV:2 * UV])
    wi8qk = np.ascontiguousarray(wi8[:, :, 2 * UV:][:, :, perm])
    wo8 = np.ascontiguousarray(
        (16.0 * Wo).reshape(UT, P, DIM).transpose(1, 0, 2)
    ).astype(ml_dtypes.float8_e4m3)

    c = float(KEY ** -0.5)
    gbb = np.stack(
        [
            np.asarray(q_gamma, np.float32)[perm] * c,
            np.asarray(q_beta, np.float32)[perm] * c,
            np.asarray(k_gamma, np.float32)[perm],
            np.asarray(k_beta, np.float32)[perm],
            bi[2 * UV:][perm],
        ],
        axis=1,
    ).astype(np.float32)
    bu = np.ascontiguousarray(
        bi[:UV].reshape(UT, P).T
    ).astype(np.float32)
    bv8 = (16.0 * bi[UV:2 * UV]).reshape(1, UV).astype(ml_dtypes.float8_e4m3)

    omega = 1.0 / (10000.0 ** (np.arange(HALF, dtype=np.float32) / HALF))
    ang = np.arange(SEQ, dtype=np.float32)[:, None] * omega[None, :]
    cos_t = np.cos(ang).T
    sin_t = np.sin(ang).T
    cc_full = np.concatenate([cos_t, cos_t], axis=0).astype(ml_dtypes.bfloat16)
    ss_full = np.concatenate([sin_t, sin_t], axis=0).astype(ml_dtypes.bfloat16)

    shared = {
        "wi8v": wi8v, "wi8u": wi8u, "wi8qk": wi8qk,
        "wo8": wo8, "gbb": gbb, "bu": bu, "bv8": bv8,
    }
    in_maps = []
    for core in range(NCORES):
        b, s = divmod(core, 2)
        order = np.concatenate(
            [
                np.arange(s * SLAB, (s + 1) * SLAB),
                np.arange((1 - s) * SLAB, (2 - s) * SLAB),
            ]
        )
        hb = h[b][order]
        m = dict(shared)
        m["ht8"] = np.ascontiguousarray(
            hb.T.reshape(KD, P, SEQ).transpose(1, 0, 2)
        ).astype(ml_dtypes.float8_e4m3)
        m["hres"] = np.ascontiguousarray(hb[:SLAB] + bo[None, :])
        m["cc"] = np.ascontiguousarray(cc_full[:, order])
        m["ss"] = np.ascontiguousarray(ss_full[:, order])
        in_maps.append(m)
    return in_maps


def kernel(hidden_states, Wi, bi, Wo, bo, q_gamma, q_beta, k_gamma, k_beta):
    global LAST_RESULT
    nc = _get_nc()
    in_maps = _host_prep(
        hidden_states, Wi, bi, Wo, bo, q_gamma, q_beta, k_gamma, k_beta
    )
    res = bass_utils.run_bass_kernel_spmd(
        nc,
        in_maps,
        core_ids=list(range(NCORES)),
        trace=bool(int(os.environ.get("KTRACE", "0"))),
    )
    LAST_RESULT = res
    out = np.empty((NB, SEQ, DIM), dtype=np.float32)
    for core in range(NCORES):
        b, s = divmod(core, 2)
        out[b, s * SLAB:(s + 1) * SLAB] = res.results[core]["out"].astype(
            np.float32
        )
    return out


# revision 29
# speedup vs baseline: 1.5104x; 1.0400x over previous
"""GAU (Gated Attention Unit) layer kernel for Trainium2, 8 NeuronCores.

Sharding: query-sequence-parallel within batch. 4 batches x 2 query slabs
of 2048 -> 8 cores. Each core receives the full 4096-token sequence of its
batch (rows reordered so its own query slab comes first), computes the
full-sequence K/V projection, and attention + output projection for its
own 2048 queries.

v2: all heavy lifting pre-staged on host (h pre-transposed + cast fp8,
Wi/Wo pre-cast fp8 with x16 scale), silu on the ACT engine (the silu
table exists on TRN2 even though CoreSim lacks it), per-partition biases
via the ACT bias path, fp8 DoubleRow output projection, bo folded into
the residual h on host. Per-core dataflow (fp32 PSUM accumulation):
  1a. qk = silu(h@Wi_qk + b) feature-major; gamma/beta + RoPE -> qT,kT
      (qk columns host-permuted evens-first; 1/sqrt(d) folded into
      q_gamma/q_beta host-side)
  1b. v = silu(h@Wi_v + b) token-major [tok,1536] fp8 (bias via ones
      matmul into PSUM; silu in one ACT op per 128-token row)
  1c. u = silu(h@Wi_u + b) feature-major fp8, bias via ACT bias
  2.  two query-pair phases (qph x 1024 tokens): scores^T = kT.T@qT,
      at = relu(s)^2 (ACT relu + DVE square, fp8), Av^T accumulated
      over 32 key tiles fp8-DR, g = u * Av^T fp8, out = g@Wo fp8-DR,
      o = po/65536 + (h+bo), RMS-normalize, DMA out.
"""

import os

import ml_dtypes
import numpy as np

import concourse.bass as bass
import concourse.mybir as mybir
import concourse.tile as tile
from concourse import bacc, bass_utils

P = 128
SEQ = 4096
DIM = 768
NCOL = 3200
UV = 1536
KEY = 128
HALF = 64
SLAB = 2048
KD = DIM // P        # 6 feature k-tiles
KD2 = KD // 2        # 3 DoubleRow feature pairs
KT = SEQ // P        # 32 key-token tiles
KT2 = KT // 2        # 16 DoubleRow key pairs
CH = 512
NCH = SEQ // CH      # 8 token chunks
OWN_CH = SLAB // CH  # 4 own (query) chunks
VT = UV // CH        # 3 v-column chunks
UT = UV // P         # 12 u/v feature tiles
UT2 = UT // 2        # 6 DoubleRow u pairs
NB = 4
NCORES = 8
EPS = 1e-12
QPH = 2              # query-pair phases
QPW = SLAB // QPH    # 1024 tokens per phase
OSC = 1.0 / (16.0 * SEQ)  # output descale: wo x16, at carries xSEQ

F32 = mybir.dt.float32
BF16 = mybir.dt.bfloat16
F8 = mybir.dt.float8e4
OP = mybir.AluOpType
AF = mybir.ActivationFunctionType
DR = mybir.MatmulPerfMode.DoubleRow

_cache = {}
LAST_RESULT = None

# ACT Silu/Gelu tables are broken on this stack (wrong values or exec-unit
# crash); always emit sigmoid + x*sig(x) on DVE.
CFG = {"silu": bool(int(os.environ.get("KSILU", "0")))}


def _build(cfg=None):
    cfg = {**CFG, **(cfg or {})}
    use_silu = cfg["silu"]
    nc = bacc.Bacc(
        "TRN2", target_bir_lowering=False, debug=False, num_devices=NCORES
    )

    def din(name, shape, dt):
        return nc.dram_tensor(name, list(shape), dt, kind="ExternalInput").ap()

    ht8_d = din("ht8", [P, KD, SEQ], F8)     # h pre-transposed, fp8
    wi8v_d = din("wi8v", [P, KD, UV], F8)    # 16*Wi v block
    wi8u_d = din("wi8u", [P, KD, UV], F8)    # 16*Wi u block
    wi8qk_d = din("wi8qk", [P, KD, KEY], F8)  # 16*Wi qk block, permuted
    wo8_d = din("wo8", [P, UT, DIM], F8)     # 16*Wo
    hres_d = din("hres", [SLAB, DIM], F32)   # own-slab h + bo
    # output in bf16 (residual dominates; host casts back to f32)
    cc_d = din("cc", [P, SEQ], BF16)
    ss_d = din("ss", [P, SEQ], BF16)
    gbb_d = din("gbb", [P, 5], F32)          # qg*c, qb*c, kg, kb, b_qk
    bu_d = din("bu", [P, UT], F32)           # bi_u per-partition
    bv8_d = din("bv8", [1, UV], F8)          # 16*bi_v
    out_d = nc.dram_tensor("out", [SLAB, DIM], BF16, kind="ExternalOutput").ap()

    def silu_act(out, in_, bias=0.0, scale=1.0, pool=None, shape=None):
        """silu from PSUM: single ACT op if the table is available, else
        sigmoid on ACT + x*sig on DVE (CoreSim fallback)."""
        if use_silu:
            nc.scalar.activation(
                out=out, in_=in_, func=AF.Silu, bias=bias, scale=scale
            )
        else:
            n = shape[-1] * (shape[1] if len(shape) > 2 else 1)
            sg = pool.tile(list(shape), BF16, tag=f"sg{n}", name="sg", bufs=2)
            nc.scalar.activation(
                out=sg, in_=in_, func=AF.Sigmoid, bias=bias, scale=scale
            )
            if isinstance(bias, float) and bias == 0.0:
                nc.vector.scalar_tensor_tensor(
                    out=out, in0=in_, scalar=scale, in1=sg,
                    op0=OP.mult, op1=OP.mult,
                )
            else:
                xx = pool.tile(list(shape), BF16, tag=f"xx{n}", name="xx",
                               bufs=2)
                nc.vector.tensor_scalar(
                    out=xx, in0=in_, scalar1=scale, scalar2=bias,
                    op0=OP.mult, op1=OP.add,
                )
                nc.vector.tensor_mul(out=out, in0=xx, in1=sg)

    with tile.TileContext(nc) as tc:
        with (
            tc.tile_pool(name="consts", bufs=1) as consts,
            tc.tile_pool(name="persist", bufs=1) as persist,
            tc.tile_pool(name="work", bufs=2) as work,
        ):
            gbb_sb = consts.tile([P, 5], F32, tag="gbb", name="gbb_sb")
            bu_sb = consts.tile([P, UT], F32, tag="bu", name="bu_sb")
            bv8_sb = consts.tile([1, UV], F8, tag="bv8", name="bv8_sb")
            ones8_sb = consts.tile([1, P], F8, tag="ones8", name="ones8_sb")
            eps_sb = consts.tile([P, 1], F32, tag="eps", name="eps_sb")
            nc.sync.dma_start(out=gbb_sb, in_=gbb_d)
            nc.sync.dma_start(out=bu_sb, in_=bu_d)
            nc.sync.dma_start(out=bv8_sb, in_=bv8_d)
            nc.vector.memset(ones8_sb, 1.0)
            nc.vector.memset(eps_sb, EPS)

            p1_cm = tc.tile_pool(name="p1", bufs=1)
            p1 = p1_cm.__enter__()
            ht8 = p1.tile([P, KD, SEQ], F8, tag="ht8", name="ht8")
            wi8v = p1.tile([P, KD, UV], F8, tag="wi8v", name="wi8v")
            wi8u = p1.tile([P, KD, UV], F8, tag="wi8u", name="wi8u")
            wi8qk = p1.tile([P, KD, KEY], F8, tag="wi8qk", name="wi8qk")
            cc_sb = p1.tile([P, SEQ], BF16, tag="cc", name="cc_sb")
            ss_sb = p1.tile([P, SEQ], BF16, tag="ss", name="ss_sb")
            # DMA order matters: v's operands first so PE can start early
            nc.sync.dma_start(out=ht8, in_=ht8_d)
            nc.sync.dma_start(out=wi8v, in_=wi8v_d)
            nc.sync.dma_start(out=wi8qk, in_=wi8qk_d)
            nc.sync.dma_start(out=cc_sb, in_=cc_d)
            nc.sync.dma_start(out=ss_sb, in_=ss_d)
            nc.sync.dma_start(out=wi8u, in_=wi8u_d)

            v8 = persist.tile([P, KT, UV], F8, tag="v8", name="v8")
            kT_sb = persist.tile([P, SEQ], BF16, tag="kT", name="kT_sb")
            qT_sb = persist.tile([P, SLAB], BF16, tag="qT", name="qT_sb")
            u8 = persist.tile([P, UT, SLAB], F8, tag="u8", name="u8")
            wo8 = persist.tile([P, UT, DIM], F8, tag="wo8", name="wo8")
            nc.sync.dma_start(out=wo8, in_=wo8_d)

            # ---- 1a+1b: v (token-major, full seq) with qk chunks
            # interleaved every 8 token-tiles so the rope DVE work
            # overlaps v's PE-heavy phase and kT/qT finish early ----
            def emit_qk(pqp, chp):
                pq = pqp.tile([P, 2, CH], F32, tag="pq", name="pq")
                for kd2 in range(KD2):
                    for chl in range(2):
                        nc.tensor.matmul(
                            pq[:, chl, :],
                            wi8qk[:, 2 * kd2:2 * kd2 + 2, :],
                            ht8[:, 2 * kd2:2 * kd2 + 2,
                                (2 * chp + chl) * CH:(2 * chp + chl + 1) * CH],
                            start=(kd2 == 0), stop=(kd2 == KD2 - 1),
                            perf_mode=DR,
                        )
                for chl in range(2):
                        ch = 2 * chp + chl
                        t0 = ch * CH
                        qk_f = work.tile([P, CH], BF16, tag="qkf", name="qk_f",
                                         bufs=2)
                        silu_act(qk_f, pq[:, chl, :], bias=gbb_sb[:, 4:5],
                                 scale=1.0 / 16, pool=work, shape=[P, CH])
                        targets = [(kT_sb[:, t0:t0 + CH], 2)]
                        if ch < OWN_CH:
                            targets.append((qT_sb[:, t0:t0 + CH], 0))
                        for dst, gi in targets:
                            pre = work.tile([P, CH], BF16, tag="pre",
                                            name="pre", bufs=2)
                            nc.vector.tensor_scalar(
                                out=pre, in0=qk_f,
                                scalar1=gbb_sb[:, gi:gi + 1],
                                scalar2=gbb_sb[:, gi + 1:gi + 2],
                                op0=OP.mult, op1=OP.add,
                            )
                            x1 = pre[0:HALF, :]
                            x2 = pre[HALF:P, :]
                            ta = work.tile([HALF, CH], BF16, tag="ta",
                                           name="ta", bufs=2)
                            tb = work.tile([HALF, CH], BF16, tag="tb",
                                           name="tb", bufs=2)
                            nc.vector.tensor_mul(
                                out=ta, in0=x1, in1=cc_sb[0:HALF, t0:t0 + CH]
                            )
                            nc.vector.tensor_mul(
                                out=tb, in0=x2, in1=ss_sb[HALF:P, t0:t0 + CH]
                            )
                            nc.vector.tensor_sub(
                                out=dst[0:HALF, :], in0=ta, in1=tb
                            )
                            tc_ = work.tile([HALF, CH], BF16, tag="ta",
                                            name="tc_", bufs=2)
                            td = work.tile([HALF, CH], BF16, tag="tb",
                                           name="td", bufs=2)
                            nc.vector.tensor_mul(
                                out=tc_, in0=x1, in1=ss_sb[0:HALF, t0:t0 + CH]
                            )
                            nc.vector.tensor_mul(
                                out=td, in0=x2, in1=cc_sb[HALF:P, t0:t0 + CH]
                            )
                            nc.vector.tensor_add(
                                out=dst[HALF:P, :], in0=tc_, in1=td
                            )

            with (
                tc.tile_pool(name="pv", bufs=2, space="PSUM") as pvp,
                tc.tile_pool(name="pq", bufs=1, space="PSUM") as pqp,
            ):
                for tt in range(KT):
                    pv = pvp.tile([P, VT, CH], F32, tag="pv", name="pv")
                    for vc in range(VT):
                        nc.tensor.matmul(
                            pv[:, vc, :], ones8_sb,
                            bv8_sb[:, vc * CH:(vc + 1) * CH],
                            start=True, stop=False,
                        )
                    for kd2 in range(KD2):
                        for vc in range(VT):
                            nc.tensor.matmul(
                                pv[:, vc, :],
                                ht8[:, 2 * kd2:2 * kd2 + 2,
                                    tt * P:(tt + 1) * P],
                                wi8v[:, 2 * kd2:2 * kd2 + 2,
                                     vc * CH:(vc + 1) * CH],
                                start=False, stop=(kd2 == KD2 - 1),
                                perf_mode=DR,
                            )
                    silu_act(v8[:, tt, :], pv[:, :, :], scale=1.0 / 16,
                             pool=work, shape=[P, UV])
                    if tt % 8 == 7:
                        emit_qk(pqp, tt // 8)

            # ---- 1c: u feature-major for own tokens ----
            with tc.tile_pool(name="pu", bufs=2, space="PSUM") as pup:
                for ut in range(UT):
                    pu = pup.tile([P, OWN_CH, CH], F32, tag="pu", name="pu")
                    for kd2 in range(KD2):
                        for qc in range(OWN_CH):
                            nc.tensor.matmul(
                                pu[:, qc, :],
                                wi8u[:, 2 * kd2:2 * kd2 + 2,
                                     ut * P:(ut + 1) * P],
                                ht8[:, 2 * kd2:2 * kd2 + 2,
                                    qc * CH:(qc + 1) * CH],
                                start=(kd2 == 0), stop=(kd2 == KD2 - 1),
                                perf_mode=DR,
                            )
                    silu_act(u8[:, ut, :], pu[:, :, :],
                             bias=bu_sb[:, ut:ut + 1], scale=1.0 / 16,
                             pool=work, shape=[P, SLAB])

            # ---- 2: attention + output, two query-pair phases ----
            p1_cm.__exit__(None, None, None)
            with (
                tc.tile_pool(name="p2", bufs=1) as p2,
                tc.tile_pool(name="ps_s", bufs=2, space="PSUM") as ps_s,
                tc.tile_pool(name="ps_av", bufs=2, space="PSUM") as ps_av,
                tc.tile_pool(name="ps_o", bufs=2, space="PSUM") as ps_o,
            ):
                at8 = p2.tile([P, KT, QPW], F8, tag="at8", name="at8")
                g8 = p2.tile([P, UT, QPW], F8, tag="g8", name="g8")
                for qph in range(QPH):
                  for qcl in range(2):
                    qg0 = qph * QPW
                    qc0 = qg0 + qcl * CH
                    # scores + relu^2 for this 512-query chunk
                    for kt in range(KT):
                        ps = ps_s.tile([P, CH], F32, tag="ps", name="ps")
                        nc.tensor.matmul(
                            ps, kT_sb[:, kt * P:(kt + 1) * P],
                            qT_sb[:, qc0:qc0 + CH],
                            start=True, stop=True,
                        )
                        rl = work.tile([P, CH], BF16, tag="rl", name="rl",
                                       bufs=3)
                        nc.scalar.activation(out=rl, in_=ps, func=AF.Relu)
                        nc.vector.tensor_mul(
                            out=at8[:, kt, qcl * CH:(qcl + 1) * CH],
                            in0=rl, in1=rl,
                        )
                    # Av accumulation
                    for ut in range(UT):
                        pav = ps_av.tile([P, CH], F32, tag="pav", name="pav")
                        for kt2 in range(KT2):
                            nc.tensor.matmul(
                                pav,
                                v8[:, 2 * kt2:2 * kt2 + 2,
                                   ut * P:(ut + 1) * P],
                                at8[:, 2 * kt2:2 * kt2 + 2,
                                    qcl * CH:(qcl + 1) * CH],
                                start=(kt2 == 0), stop=(kt2 == KT2 - 1),
                                perf_mode=DR,
                            )
                        nc.vector.scalar_tensor_tensor(
                            out=g8[:, ut, qcl * CH:(qcl + 1) * CH],
                            in0=pav, scalar=1.0,
                            in1=u8[:, ut, qc0:qc0 + CH],
                            op0=OP.mult, op1=OP.mult,
                        )
                    # output projection + residual + RMS norm
                    for tl in range(CH // P):
                        tok_l = qcl * CH + tl * P
                        tok_g = qg0 + tok_l
                        po_a = ps_o.tile([P, CH], F32, tag="poa", name="po_a")
                        po_b = ps_o.tile([P, DIM - CH], F32, tag="pob",
                                         name="po_b")
                        for u2 in range(UT2):
                            g_t = g8[:, 2 * u2:2 * u2 + 2, tok_l:tok_l + P]
                            nc.tensor.matmul(
                                po_a, g_t, wo8[:, 2 * u2:2 * u2 + 2, 0:CH],
                                start=(u2 == 0), stop=(u2 == UT2 - 1),
                                perf_mode=DR,
                            )
                            nc.tensor.matmul(
                                po_b, g_t, wo8[:, 2 * u2:2 * u2 + 2, CH:DIM],
                                start=(u2 == 0), stop=(u2 == UT2 - 1),
                                perf_mode=DR,
                            )
                        hres = work.tile([P, DIM], F32, tag="hres",
                                         name="hres", bufs=2)
                        nc.sync.dma_start(
                            out=hres, in_=hres_d[tok_g:tok_g + P, :]
                        )
                        o_sb = work.tile([P, DIM], F32, tag="osb",
                                         name="o_sb", bufs=2)
                        nc.vector.scalar_tensor_tensor(
                            out=o_sb[:, 0:CH], in0=po_a, scalar=OSC,
                            in1=hres[:, 0:CH], op0=OP.mult, op1=OP.add,
                        )
                        nc.vector.scalar_tensor_tensor(
                            out=o_sb[:, CH:DIM], in0=po_b, scalar=OSC,
                            in1=hres[:, CH:DIM], op0=OP.mult, op1=OP.add,
                        )
                        o2 = work.tile([P, DIM], F8, tag="o2", name="o2",
                                       bufs=2)
                        ms = work.tile([P, 1], F32, tag="ms", name="ms")
                        nc.scalar.activation(
                            out=o2, in_=o_sb, func=AF.Square, accum_out=ms
                        )
                        sd = work.tile([P, 1], F32, tag="sd", name="sd")
                        nc.scalar.activation(
                            out=sd, in_=ms, func=AF.Sqrt,
                            bias=eps_sb[:, 0:1], scale=1.0 / DIM,
                        )
                        rinv = work.tile([P, 1], F32, tag="rinv", name="rinv")
                        nc.vector.reciprocal(out=rinv, in_=sd)
                        ofin = work.tile([P, DIM], BF16, tag="ofin",
                                         name="ofin", bufs=2)
                        nc.scalar.mul(ofin, o_sb, rinv[:, 0:1])
                        nc.sync.dma_start(
                            out=out_d[tok_g:tok_g + P, :], in_=ofin
                        )
    nc.compile()
    return nc


def _get_nc(cfg=None):
    key = ("nc", tuple(sorted((cfg or CFG).items())))
    if key not in _cache:
        _cache[key] = _build(cfg)
    return _cache[key]


def _host_prep(hidden_states, Wi, bi, Wo, bo, q_gamma, q_beta, k_gamma, k_beta):
    h = np.ascontiguousarray(np.asarray(hidden_states, dtype=np.float32))
    Wi = np.asarray(Wi, dtype=np.float32)
    bi = np.asarray(bi, dtype=np.float32)
    Wo = np.asarray(Wo, dtype=np.float32)
    bo = np.asarray(bo, dtype=np.float32)

    perm = np.concatenate([np.arange(0, KEY, 2), np.arange(1, KEY, 2)])
    # scale into e4m3 normal range; 1/16 applied after psum
    wi8 = np.ascontiguousarray(
        (16.0 * Wi).reshape(KD, P, NCOL).transpose(1, 0, 2)
    ).astype(ml_dtypes.float8_e4m3)
    wi8u = np.ascontiguousarray(wi8[:, :, :UV])
    wi8v = np.ascontiguousarray(wi8[:, :, UV:2 * UV])
    wi8qk = np.ascontiguousarray(wi8[:, :, 2 * UV:][:, :, perm])
    wo8 = np.ascontiguousarray(
        (16.0 * Wo).reshape(UT, P, DIM).transpose(1, 0, 2)
    ).astype(ml_dtypes.float8_e4m3)

    c = float(KEY ** -0.5)
    gbb = np.stack(
        [
            np.asarray(q_gamma, np.float32)[perm] * c,
            np.asarray(q_beta, np.float32)[perm] * c,
            np.asarray(k_gamma, np.float32)[perm],
            np.asarray(k_beta, np.float32)[perm],
            bi[2 * UV:][perm],
        ],
        axis=1,
    ).astype(np.float32)
    bu = np.ascontiguousarray(
        bi[:UV].reshape(UT, P).T
    ).astype(np.float32)
    bv8 = (16.0 * bi[UV:2 * UV]).reshape(1, UV).astype(ml_dtypes.float8_e4m3)

    omega = 1.0 / (10000.0 ** (np.arange(HALF, dtype=np.float32) / HALF))
    ang = np.arange(SEQ, dtype=np.float32)[:, None] * omega[None, :]
    cos_t = np.cos(ang).T
    sin_t = np.sin(ang).T
    cc_full = np.concatenate([cos_t, cos_t], axis=0).astype(ml_dtypes.bfloat16)
    ss_full = np.concatenate([sin_t, sin_t], axis=0).astype(ml_dtypes.bfloat16)

    shared = {
        "wi8v": wi8v, "wi8u": wi8u, "wi8qk": wi8qk,
        "wo8": wo8, "gbb": gbb, "bu": bu, "bv8": bv8,
    }
    in_maps = []
    for core in range(NCORES):
        b, s = divmod(core, 2)
        order = np.concatenate(
            [
                np.arange(s * SLAB, (s + 1) * SLAB),
                np.arange((1 - s) * SLAB, (2 - s) * SLAB),
            ]
        )
        hb = h[b][order]
        m = dict(shared)
        m["ht8"] = np.ascontiguousarray(
            hb.T.reshape(KD, P, SEQ).transpose(1, 0, 2)
        ).astype(ml_dtypes.float8_e4m3)
        m["hres"] = np.ascontiguousarray(hb[:SLAB] + bo[None, :])
        m["cc"] = np.ascontiguousarray(cc_full[:, order])
        m["ss"] = np.ascontiguousarray(ss_full[:, order])
        in_maps.append(m)
    return in_maps


def kernel(hidden_states, Wi, bi, Wo, bo, q_gamma, q_beta, k_gamma, k_beta):
    global LAST_RESULT
    nc = _get_nc()
    in_maps = _host_prep(
        hidden_states, Wi, bi, Wo, bo, q_gamma, q_beta, k_gamma, k_beta
    )
    res = bass_utils.run_bass_kernel_spmd(
        nc,
        in_maps,
        core_ids=list(range(NCORES)),
        trace=bool(int(os.environ.get("KTRACE", "0"))),
    )
    LAST_RESULT = res
    out = np.empty((NB, SEQ, DIM), dtype=np.float32)
    for core in range(NCORES):
        b, s = divmod(core, 2)
        out[b, s * SLAB:(s + 1) * SLAB] = res.results[core]["out"].astype(
            np.float32
        )
    return out
